# revision 11
# baseline (speedup 1.0000x reference)
# Trainium2 Bass kernel for nn_Encoder_81509889343552.
# Encoder-decoder CNN (7x7 conv -> 4 stride-2 convs -> 4 stride-2 convT -> 7x7
# conv + InstanceNorm/ReLU, tanh) followed by a masked segment mean.
#
# Sharding: H-split SPMD over 8 cores. Core c (0-3) computes the TOP half of
# image c; core c+4 the BOTTOM half. Per layer boundary one pairwise AllGather
# exchanges (a) per-channel partial InstanceNorm stats [mean, E[x^2]] (exact
# global stats = average of the two halves) and (b) the raw boundary row(s)
# needed as conv halos. The program is identical on all cores; top/bottom
# asymmetry is data-driven via 0/1 mask inputs.
#
# All conv matmuls run in bf16 (full PE rate), accumulating fp32 in PSUM.
# Activations are stored in HBM as bf16 in consumer-friendly layouts with
# baked-in zero gutter columns so every DMA is one contiguous chunk per
# partition. Weights are pre-transposed on the host. The final 7x7 conv
# computes output columns 3..508 on device (no column reflect needed); the 6
# reflected edge columns are produced by a small batched matmul pass over
# saved stage-A partials. The masked segment mean is finished on the host
# from the per-core tanh outputs.
import sys

sys.path.insert(0, "/opt/trn_rl_repo")

import contextlib

import numpy as np
import ml_dtypes

import concourse.bass as bass
import concourse.bacc as bacc
import concourse.tile as tile
from concourse import mybir
from concourse.bass_utils import run_bass_kernel_spmd

F32 = mybir.dt.float32
BF16 = mybir.dt.bfloat16
AF = mybir.ActivationFunctionType
ALU = mybir.AluOpType
BF = ml_dtypes.bfloat16

B, H, W = 4, 512, 512
EPS = 1e-5
P = 128
OWN = 256  # h0 rows owned per core
PAIRS = [[0, 4], [1, 5], [2, 6], [3, 7]]

# down layers: (Cin, Cout, Wi, own_out, nr, nrc)
DCFG = [(64, 128, 512, 128, 8, 2),
        (128, 256, 256, 64, 8, 4),
        (256, 512, 128, 32, 8, 8),
        (512, 1024, 64, 16, 16, 16)]
# up layers 0-2: (Cin, Cout, Wi, own_out, nr, rsub)
UCFG = [(1024, 512, 32, 32, 32, 16),
        (512, 256, 64, 64, 16, 8),
        (256, 128, 128, 128, 8, 4)]
# u3: 128 -> 64, Wi=256, own_out=256, nr=4, rsub=2

Y0LIST = list(range(0, 250, 6)) + [250]  # 43 final-conv strips per core


def _ap(base, extra_off, dims):
    return bass.AP(tensor=base.tensor, offset=base.offset + extra_off,
                   ap=[list(base.ap[0])] + [list(d) for d in dims])


def _dap(handle, off, dims):
    return bass.AP(tensor=handle, offset=off, ap=[list(d) for d in dims])


# ----------------------------------------------------------------------------
# Host-side weight preprocessing (all blobs bf16, K-partition-major)
# ----------------------------------------------------------------------------

def prep_weights(inp):
    w = {}
    # L1 7x7: K=96: p = ci*32 + dy*4 + l (dy 0..7, l 0..3); matmul d (0..1)
    # covers kx = 4d + l; M = r*64 + co (r 0..1).
    w0 = np.asarray(inp["w0"], np.float32)  # [64, 3, 7, 7]
    w1 = np.zeros((96, 2, 128), np.float32)
    for ci in range(3):
        for dy in range(8):
            for l in range(4):
                p = ci * 32 + dy * 4 + l
                for d in range(2):
                    kx = 4 * d + l
                    if kx > 6:
                        continue
                    for r in range(2):
                        ky = dy - r
                        if 0 <= ky <= 6:
                            w1[p, d, r * 64:(r + 1) * 64] = w0[:, ci, ky, kx]
    w["w1"] = w1.astype(BF)

    # down convs: [K, cbo, cbi, 3, 3, 128]
    c = 64
    for i in range(4):
        dw = np.asarray(inp[f"dw{i}"], np.float32)  # [2c, c, 3, 3]
        cbo, cbi, K = (2 * c) // P, max(c // P, 1), min(c, P)
        blob = np.zeros((K, cbo, cbi, 3, 3, P), np.float32)
        for m in range(cbo):
            for cb in range(cbi):
                blob[:, m, cb] = dw[m * P:(m + 1) * P,
                                    cb * K:(cb + 1) * K].transpose(1, 2, 3, 0)
        w[f"wd{i}"] = blob.astype(BF)
        c *= 2

    # up convs (torch convT layout uw [Cin, Cout, 3, 3]): [128, cbo, cbi, 3, 3, Mo]
    for i in range(4):
        uw = np.asarray(inp[f"uw{i}"], np.float32)
        Cin_, Cout_ = uw.shape[0], uw.shape[1]
        cbi, cbo, Mo = Cin_ // P, max(Cout_ // P, 1), min(Cout_, P)
        blob = np.zeros((P, cbo, cbi, 3, 3, Mo), np.float32)
        for m in range(cbo):
            for cb in range(cbi):
                blob[:, m, cb] = uw[cb * P:(cb + 1) * P,
                                    m * Mo:(m + 1) * Mo].transpose(0, 2, 3, 1)
        w[f"wu{i}"] = blob.astype(BF)
        c //= 2

    # final 7x7 stage A: K = j*64+ch, M = r*21 + dx*3 + co; round t reads
    # padded rows y0+2t+j => ky = 2t + j - r.
    wf = np.asarray(inp["wf"], np.float32)  # [3, 64, 7, 7]
    wfA = np.zeros((128, 6, 126), np.float32)
    for t in range(6):
        for j in range(2):
            for r in range(6):
                ky = 2 * t + j - r
                if 0 <= ky <= 6:
                    for dx in range(7):
                        for co in range(3):
                            wfA[j * 64:(j + 1) * 64, t, r * 21 + dx * 3 + co] = \
                                wf[co, :, ky, dx]
    # stage B gather: [126, 7, 18]
    wfS = np.zeros((126, 7, 18), np.float32)
    for dx in range(7):
        for r in range(6):
            for co in range(3):
                wfS[r * 21 + dx * 3 + co, dx, r * 3 + co] = 1.0
    w["wfA"] = wfA.astype(BF)
    w["wfS"] = wfS.astype(BF)
    bf = np.asarray(inp["bf"], np.float32)
    w["bfv"] = np.tile(bf, 6).reshape(18, 1).astype(np.float32)

    # L1 stats r-fold: average partition pairs (r*64+co) -> co
    wfold = np.zeros((128, 64), np.float32)
    for r in range(2):
        for co in range(64):
            wfold[r * 64 + co, co] = 0.5
    w["wfold"] = wfold.astype(BF)
    return w


def prep_core_inputs(x_img, is_bottom, wblobs):
    # xrep: [96, 128, 518] bf16 per core; row-pair index s covers h0 rows
    # (2s, 2s+1) in core-local coords; p = ci*32 + dy*4 + l reads
    # xpad[ci, 2s + base + dy, l : l+515...].
    xpad = np.pad(np.asarray(x_img, np.float32), ((0, 0), (3, 3), (3, 3)),
                  mode="reflect")  # [3, 518, 518]
    base = 256 if is_bottom else 0
    xrep = np.zeros((96, 128, 518), np.float32)
    for ci in range(3):
        for dy in range(8):
            for l in range(4):
                p = ci * 32 + dy * 4 + l
                rows = xpad[ci, base + dy: base + dy + 256: 2, l:]
                xrep[p, :, :rows.shape[1]] = rows
    m = {"xrep": xrep.astype(BF),
         "maskb": np.full((P, 1), float(is_bottom), np.float32),
         "maskt": np.full((P, 1), float(not is_bottom), np.float32)}
    m.update(wblobs)
    return m


# ----------------------------------------------------------------------------
# Device kernel
# ----------------------------------------------------------------------------

def build_kernel(debug=False):
    nc = bacc.Bacc(None, target_bir_lowering=False)

    xrep = nc.dram_tensor("xrep", [96, 128, 518], BF16, kind="ExternalInput")
    maskb_h = nc.dram_tensor("maskb", [P, 1], F32, kind="ExternalInput")
    maskt_h = nc.dram_tensor("maskt", [P, 1], F32, kind="ExternalInput")
    w1 = nc.dram_tensor("w1", [96, 2, 128], BF16, kind="ExternalInput")
    wd, wu = [], []
    c = 64
    for i in range(4):
        cbo, cbi, K = (2 * c) // P, max(c // P, 1), min(c, P)
        wd.append(nc.dram_tensor(f"wd{i}", [K, cbo, cbi, 3, 3, P], BF16,
                                 kind="ExternalInput"))
        c *= 2
    for i in range(4):
        cbi, cbo, Mo = c // P, max((c // 2) // P, 1), min(c // 2, P)
        wu.append(nc.dram_tensor(f"wu{i}", [P, cbo, cbi, 3, 3, Mo], BF16,
                                 kind="ExternalInput"))
        c //= 2
    wfA = nc.dram_tensor("wfA", [128, 6, 126], BF16, kind="ExternalInput")
    wfS = nc.dram_tensor("wfS", [126, 7, 18], BF16, kind="ExternalInput")
    bfv = nc.dram_tensor("bfv", [18, 1], F32, kind="ExternalInput")
    wfold = nc.dram_tensor("wfold", [128, 64], BF16, kind="ExternalInput")

    # activations (bf16). Down-consumed: [cb][128][slots][W+1], col 0 = zero
    # gutter, slot 0 = halo-above. Up-consumed: data cols 0..W-1, col W = 0,
    # last slot = halo-below.
    h0 = nc.dram_tensor("h0", [64, 257, 513], BF16)
    h1 = nc.dram_tensor("h1", [1, P, 129, 257], BF16)
    h2 = nc.dram_tensor("h2", [2, P, 65, 129], BF16)
    h3 = nc.dram_tensor("h3", [4, P, 33, 65], BF16)
    h4 = nc.dram_tensor("h4", [8, P, 17, 33], BF16)
    g0 = nc.dram_tensor("g0", [4, P, 33, 65], BF16)
    g1 = nc.dram_tensor("g1", [2, P, 65, 129], BF16)
    g2 = nc.dram_tensor("g2", [1, P, 129, 257], BF16)
    # g3 split by padded-row parity, unpadded width: rows = padded 0..261
    g3e = nc.dram_tensor("g3e", [64, 131, 512], BF16)  # even padded rows
    g3o = nc.dram_tensor("g3o", [64, 131, 512], BF16)  # odd padded rows
    dsts_down = [h1, h2, h3, h4]
    srcs_up = [h4, g0, g1, g2]

    # collective buffers (bf16 payload; stats are fp32 bitcast into 4 cols)
    ccin, ccout = {}, {}
    for name, cbo, Wc, nrows in [("b0", 1, 512, 1), ("b1", 1, 256, 1),
                                 ("b2", 2, 128, 1), ("b3", 4, 64, 1),
                                 ("b4", 8, 32, 1), ("b5", 4, 64, 1),
                                 ("b6", 2, 128, 1), ("b7", 1, 256, 1),
                                 ("b8", 1, 512, 3)]:
        F = 4 + nrows * Wc * (2 if name == "b8" else 1)
        ccin[name] = nc.dram_tensor(f"ccin_{name}", [cbo, P, F], BF16)
        ccout[name] = nc.dram_tensor(f"ccout_{name}", [2, cbo, P, F], BF16)

    hfout = nc.dram_tensor("hfout", [18, 43, 506], F32, kind="ExternalOutput")
    efout = nc.dram_tensor("efout", [18, 6, 43], F32, kind="ExternalOutput")

    with tile.TileContext(nc) as tc, contextlib.ExitStack() as ctx:
        sb = ctx.enter_context(tc.tile_pool(name="sb", bufs=3))
        wsm = ctx.enter_context(tc.tile_pool(name="wsm", bufs=2))
        nrm = ctx.enter_context(tc.tile_pool(name="nrm", bufs=1))
        stp = ctx.enter_context(tc.tile_pool(name="stp", bufs=1))
        ps = ctx.enter_context(tc.tile_pool(name="ps", bufs=3, space="PSUM"))
        psf = ctx.enter_context(tc.tile_pool(name="psf", bufs=2, space="PSUM"))

        eps_t = nrm.tile([P, 1], F32, name="eps_t")
        nc.vector.memset(eps_t, EPS)
        maskb = nrm.tile([P, 1], F32, name="maskb_t")
        nc.sync.dma_start(out=maskb, in_=maskb_h[:, :])
        maskt = nrm.tile([P, 1], F32, name="maskt_t")
        nc.sync.dma_start(out=maskt, in_=maskt_h[:, :])

        def finalize_stats_batch(meg, npart, cbo, name):
            """meg [npart, cbo, 2] = (mean, E[x^2]) -> [(scale, bias)] APs."""
            s_t = nrm.tile([npart, cbo, 2], F32, name=f"st_{name}",
                           tag=f"st_{name}")
            var = stp.tile([npart, cbo, 2], F32, name=f"var_{name}",
                           tag="stmp2")
            nc.vector.tensor_mul(out=var[:, :, 0:1], in0=meg[:, :, 0:1],
                                 in1=meg[:, :, 0:1])
            nc.vector.tensor_sub(out=var[:, :, 1:2], in0=meg[:, :, 1:2],
                                 in1=var[:, :, 0:1])
            tmp = stp.tile([npart, cbo, 1], F32, name=f"tmp_{name}",
                           tag="stmp")
            nc.scalar.activation(out=tmp, in_=var[:, :, 1:2], func=AF.Sqrt,
                                 bias=eps_t[:npart], scale=1.0)
            nc.vector.reciprocal(out=s_t[:, :, 0:1], in_=tmp)
            nc.vector.tensor_mul(out=s_t[:, :, 1:2], in0=meg[:, :, 0:1],
                                 in1=s_t[:, :, 0:1])
            nc.vector.tensor_scalar(out=s_t[:, :, 1:2], in0=s_t[:, :, 1:2],
                                    scalar1=-1.0, scalar2=None, op0=ALU.mult)
            return [(s_t[:, cb, 0:1], s_t[:, cb, 1:2]) for cb in range(cbo)]

        def mv_to_me(mv_ap, me_all, m):
            """bn_aggr [mean, var] -> me_all[:, m, :] = [mean, E[x^2]]."""
            nc.vector.tensor_copy(out=me_all[:, m, 0:1], in_=mv_ap[:, 0:1])
            nc.vector.tensor_scalar(out=me_all[:, m, 1:2], in0=mv_ap[:, 0:1],
                                    scalar1=mv_ap[:, 0:1], scalar2=None,
                                    op0=ALU.mult)
            nc.vector.tensor_add(out=me_all[:, m, 1:2], in0=me_all[:, m, 1:2],
                                 in1=mv_ap[:, 1:2])

        def cc_exchange(name, me_all, row_tiles, npart):
            """Write stats+rows to ccin, AllGather.

            me_all: [npart, cbo, 2] fp32 tile. row_tiles: list of
            (bf16 tile [npart, cbo, Wrow] or [npart, Wrow], col_offset).
            """
            ci, co_ = ccin[name], ccout[name]
            F = ci.shape[2]
            cbo = me_all.shape[1]
            nc.gpsimd.dma_start(
                out=_dap(ci, 0, [[F, npart], [P * F, cbo], [1, 4]]),
                in_=me_all.bitcast(BF16).rearrange("p c f -> p (c f)"))
            for (rt, coff) in row_tiles:
                if len(rt.shape) == 3:
                    wrow = rt.shape[2]
                    nbl = rt.shape[1]
                    rt = rt.rearrange("p c w -> p (c w)")
                else:
                    wrow, nbl = rt.shape[1], 1
                nc.gpsimd.dma_start(
                    out=_dap(ci, coff,
                             [[F, npart], [P * F, nbl], [1, wrow]]),
                    in_=rt)
            nc.gpsimd.collective_compute(
                "AllGather", mybir.AluOpType.bypass, replica_groups=PAIRS,
                ins=[ci[:, :, :]], outs=[co_[:, :, :, :]])
            return co_

        def cc_stats(name, cbo, npart, sname):
            """Read gathered stats, average halves -> meg [npart, cbo, 2]."""
            co_ = ccout[name]
            F = co_.shape[3]
            sa = stp.tile([npart, 2, cbo, 4], BF16, name=f"sa_{sname}",
                          tag="ccst")
            nc.gpsimd.dma_start(
                out=sa.rearrange("p s c f -> p (s c f)"),
                in_=_dap(co_, 0, [[F, npart], [cbo * P * F, 2],
                                  [P * F, cbo], [1, 4]]))
            saf = sa.bitcast(F32)  # [npart, 2, cbo, 2]
            meg = stp.tile([npart, cbo, 2], F32, name=f"meg_{sname}",
                           tag="ccme")
            nc.vector.tensor_add(out=meg, in0=saf[:, 0], in1=saf[:, 1])
            nc.vector.tensor_scalar(out=meg, in0=meg, scalar1=0.5,
                                    scalar2=None, op0=ALU.mult)
            return meg

        def cc_halo(name, cb, npart, wrow, coff, slot_hi, mask, nbl=1):
            """Halo rows from gathered slot -> bf16 tile [npart, nbl, wrow]."""
            co_ = ccout[name]
            F = co_.shape[3]
            cbo = co_.shape[1]
            src_off = ((cbo if slot_hi else 0) + cb) * P * F + coff
            hrow = sb.tile([npart, nbl, wrow], BF16, name=f"hr_{name}",
                           tag="halo")
            nc.gpsimd.dma_start(
                out=hrow.rearrange("p c w -> p (c w)"),
                in_=_dap(co_, src_off,
                         [[F, npart], [P * F, nbl], [1, wrow]]))
            if mask is not None:
                nc.vector.tensor_scalar(out=hrow, in0=hrow,
                                        scalar1=mask[:npart],
                                        scalar2=None, op0=ALU.mult)
            return hrow

        # ================= L1: 7x7 conv, 3 -> 64 =========================
        _sc = nc.enter_named_scope("L1", False)[0]
        w1t = wsm.tile([96, 2, 128], BF16, name="w1t", tag="w1", bufs=1)
        nc.sync.dma_start(out=w1t, in_=w1[:, :, :])
        wft = wsm.tile([P, 64], BF16, name="wft", tag="wfold", bufs=1)
        nc.sync.dma_start(out=wft, in_=wfold[:, :])

        st1 = stp.tile([P, 128, 6], F32, name="st1", tag="stats")
        rl_h0 = nrm.tile([64, 512], BF16, name="rl_h0")  # last own h0 row
        NS1 = 32
        for s_i in range(NS1):
            slab = sb.tile([96, 4, 518], BF16, name="slab1", tag="inslab")
            nc.sync.dma_start(out=slab,
                              in_=_dap(xrep, s_i * 4 * 518,
                                       [[128 * 518, 96], [1, 4 * 518]]))
            oslab = sb.tile([64, 8, 513], BF16, name="oslab1", tag="outslab")
            nc.gpsimd.memset(oslab[:, :, 0:1], 0.0)
            for k in range(4):
                pt = ps.tile([P, 512], F32, name="pt1", tag="mm")
                for d in range(2):
                    rhs = _ap(slab[:, 0, 0], k * 518 + 4 * d, [[1, 512]])
                    nc.tensor.matmul(pt, w1t[:, d, :], rhs,
                                     start=(d == 0), stop=(d == 1))
                nc.vector.bn_stats(out=st1[:, s_i * 4 + k, :], in_=pt)
                nc.scalar.activation(out=oslab[:, 2 * k, 1:513],
                                     in_=pt[0:64, :], func=AF.Copy)
                nc.scalar.activation(out=oslab[:, 2 * k + 1, 1:513],
                                     in_=pt[64:128, :], func=AF.Copy)
            if s_i == NS1 - 1:
                nc.vector.tensor_copy(out=rl_h0, in_=oslab[:, 7, 1:513])
            nc.sync.dma_start(
                out=_dap(h0, (1 + s_i * 8) * 513, [[257 * 513, 64], [1, 8 * 513]]),
                in_=oslab)
        mv1 = stp.tile([P, 2], F32, name="mv1", tag="mv")
        nc.vector.bn_aggr(out=mv1, in_=st1)
        me1 = stp.tile([P, 2], F32, name="me1", tag="me")
        mv_to_me(mv1, me1)
        cc_exchange("b0", [me1], [(rl_h0, 0, 4)], P)
        # stats: average halves -> fold r-pairs via matmul -> scale/bias
        meg0 = cc_stats("b0", 1, P, "h0")[0][0]
        megb = stp.tile([P, 2], BF16, name="megb", tag="megb")
        nc.vector.tensor_copy(out=megb, in_=meg0)
        pm = ps.tile([64, 2], F32, name="pm", tag="mini", bufs=1)
        nc.tensor.matmul(pm, wft, megb, start=True, stop=True)
        me0g = stp.tile([64, 2], F32, name="me0g", tag="mvg")
        nc.scalar.activation(out=me0g, in_=pm, func=AF.Copy)
        st_h0 = [finalize_stats(me0g, 64, "h0")]
        # halo row -> h0 slot 0 (top: zeros = zero pad); post-norm + masked
        hrow = cc_halo("b0", 0, 64, 512, 4, False, None)
        hwr = sb.tile([64, 513], BF16, name="hwr0", tag="halow")
        nc.gpsimd.memset(hwr[:, 0:1], 0.0)
        nc.scalar.activation(out=hrow, in_=hrow, func=AF.Relu,
                             bias=st_h0[0][:, 1:2], scale=st_h0[0][:, 0:1])
        nc.vector.tensor_scalar(out=hwr[:, 1:513], in0=hrow,
                                scalar1=maskb[:64], scalar2=None, op0=ALU.mult)
        nc.gpsimd.dma_start(out=_dap(h0, 0, [[257 * 513, 64], [1, 513]]), in_=hwr)

        nc.leave_named_scope("L1", _sc, False)
        _sc = nc.enter_named_scope("down", False)[0]

        # ================= down convs =====================================
        def down_layer(li, src, dst, wsrc, st_in, bname):
            Cin, Cout, Wi, own_out, nr, nrc = DCFG[li]
            up_dst = (li == 3)  # h4 is consumed by an up-conv layer
            Wo = Wi // 2
            cbi, cbo, K = max(Cin // P, 1), Cout // P, min(Cin, P)
            Wp, Wq = Wi + 1, Wo + 1
            nstrip, nchunk = own_out // nr, nr // nrc
            rows_in = 2 * nr + 1
            stt = stp.tile([P, cbo, nstrip * nchunk, 6], F32,
                           name=f"std{li}", tag="stats")
            rlast = nrm.tile([P, cbo, Wo], BF16, name=f"rl_d{li}")
            gcol = slice(Wo, Wq) if up_dst else slice(0, 1)
            dcol = slice(0, Wo) if up_dst else slice(1, Wq)
            # weights resident for the whole layer
            wt = wsm.tile([K, cbo, cbi, 3, 3, P], BF16, name=f"wtd{li}",
                          tag="wshared", bufs=1)
            nc.sync.dma_start(out=wt, in_=wsrc[:, :, :, :, :, :])
            for s_i in range(nstrip):
                slab = sb.tile([K, cbi, rows_in, Wp], BF16,
                               name=f"sld{li}", tag="inslab")
                for cb in range(cbi):
                    if src is h0:
                        sap = _dap(h0, (2 * s_i * nr) * 513,
                                   [[257 * 513, 64], [1, rows_in * 513]])
                    else:
                        sap = _dap(src, cb * P * src.shape[2] * src.shape[3]
                                   + (2 * s_i * nr) * Wp,
                                   [[src.shape[2] * src.shape[3], P],
                                    [1, rows_in * Wp]])
                    nc.sync.dma_start(
                        out=slab[:, cb, :, :].rearrange("k r w -> k (r w)"),
                        in_=sap)
                    r0a = 1 if s_i == 0 else 0  # halo row is already post-norm
                    nc.scalar.activation(
                        out=slab[:, cb, r0a:, 1:Wp],
                        in_=slab[:, cb, r0a:, 1:Wp],
                        func=AF.Relu, bias=st_in[cb][:, 1:2],
                        scale=st_in[cb][:, 0:1])
                oslab = sb.tile([P, cbo, nr, Wq], BF16, name=f"osd{li}",
                                tag="outslab")
                nc.gpsimd.memset(oslab[:, :, :, gcol], 0.0)
                for m in range(cbo):
                    for chk in range(nchunk):
                        pt = ps.tile([P, nrc, Wo], F32, name=f"ptd{li}",
                                     tag="mm")
                        first = True
                        for cb in range(cbi):
                            for dy in range(3):
                                for dx in range(3):
                                    row0 = 2 * chk * nrc + dy
                                    rhs = _ap(slab[:, 0, 0, 0],
                                              cb * rows_in * Wp + row0 * Wp + dx,
                                              [[2 * Wp, nrc], [2, Wo]])
                                    last = (cb == cbi - 1 and dy == 2
                                            and dx == 2)
                                    nc.tensor.matmul(
                                        pt, wt[:, m, cb, dy, dx, :], rhs,
                                        start=first, stop=last)
                                    first = False
                        nc.vector.bn_stats(
                            out=stt[:, m, s_i * nchunk + chk, :],
                            in_=pt.rearrange("p a b -> p (a b)"))
                        nc.scalar.activation(
                            out=oslab[:, m, chk * nrc:(chk + 1) * nrc, dcol],
                            in_=pt, func=AF.Copy)
                if up_dst and s_i == 0:
                    nc.vector.tensor_copy(out=rlast,
                                          in_=oslab[:, :, 0, dcol])
                if not up_dst and s_i == nstrip - 1:
                    nc.vector.tensor_copy(out=rlast,
                                          in_=oslab[:, :, nr - 1, dcol])
                row_base = s_i * nr if up_dst else 1 + s_i * nr
                for m in range(cbo):
                    nc.sync.dma_start(
                        out=_dap(dst, m * P * dst.shape[2] * dst.shape[3]
                                 + row_base * Wq,
                                 [[dst.shape[2] * dst.shape[3], P],
                                  [1, nr * Wq]]),
                        in_=oslab[:, m, :, :].rearrange("p r w -> p (r w)"))
            # stats + boundary exchange
            mes = []
            for m in range(cbo):
                mv = stp.tile([P, 2], F32, name=f"mvd{li}", tag="mv")
                nc.vector.bn_aggr(out=mv, in_=stt[:, m, :, :])
                me = stp.tile([P, 2], F32, name=f"med{li}m{m}", tag=f"me{li}{m}")
                mv_to_me(mv, me)
                mes.append(me)
            cc_exchange(bname, mes,
                        [(rlast[:, m, :], m, 4) for m in range(cbo)], P)
            st_outs = []
            for meg, cb in cc_stats(bname, cbo, P, f"d{li}"):
                st_outs.append(finalize_stats(meg, P, f"d{li}m{cb}"))
            # halo row (down-style: peer last row -> slot 0, masked maskb;
            # up-style dst: peer first row -> last slot, masked maskt)
            for m in range(cbo):
                hr = cc_halo(bname, m, P, Wo, 4, up_dst, None)
                hw = sb.tile([P, Wq], BF16, name=f"hwd{li}", tag="halow")
                nc.gpsimd.memset(hw[:, gcol], 0.0)
                nc.scalar.activation(out=hr, in_=hr, func=AF.Relu,
                                     bias=st_outs[m][:, 1:2],
                                     scale=st_outs[m][:, 0:1])
                nc.vector.tensor_scalar(out=hw[:, dcol], in0=hr,
                                        scalar1=maskt if up_dst else maskb,
                                        scalar2=None, op0=ALU.mult)
                halo_row = dst.shape[2] - 1 if up_dst else 0
                nc.gpsimd.dma_start(
                    out=_dap(dst, m * P * dst.shape[2] * dst.shape[3]
                             + halo_row * Wq,
                             [[dst.shape[2] * dst.shape[3], P], [1, Wq]]),
                    in_=hw)
            return st_outs

        st_h1 = down_layer(0, h0, h1, wd[0], st_h0, "b1")
        st_h2 = down_layer(1, h1, h2, wd[1], st_h1, "b2")
        st_h3 = down_layer(2, h2, h3, wd[2], st_h2, "b3")
        st_h4 = down_layer(3, h3, h4, wd[3], st_h3, "b4")

        nc.leave_named_scope("down", _sc, False)
        _sc = nc.enter_named_scope("up", False)[0]

        # ================= up convs 0-2 ===================================
        def up_layer(li, src, dst, wsrc, st_in, bname):
            Cin, Cout, Wi, own_out, nr, rsub = UCFG[li]
            Wo = 2 * Wi
            cbi, cbo, Mo = Cin // P, Cout // P, P
            Wp, Wq = Wi + 1, Wo + 1
            nstrip = own_out // nr
            n_cr = nr // 2
            nsub = n_cr // rsub
            srlen = src.shape[2] * src.shape[3]
            drlen = dst.shape[2] * dst.shape[3]
            stt = stp.tile([P, cbo, nstrip * 4 * nsub, 6], F32,
                           name=f"stu{li}", tag="stats")
            rfirst = nrm.tile([P, cbo, Wo], BF16, name=f"rf_u{li}")
            wt = wsm.tile([P, cbo, cbi, 3, 3, Mo], BF16, name=f"wtu{li}",
                          tag="wshared", bufs=1)
            nc.sync.dma_start(out=wt, in_=wsrc[:, :, :, :, :, :])
            for s_i in range(nstrip):
                y0 = s_i * nr
                i_lo = y0 // 2
                rows_in = nr // 2 + 1
                slab = sb.tile([P, cbi, rows_in, Wp], BF16,
                               name=f"slu{li}", tag="inslab")
                for cb in range(cbi):
                    nc.sync.dma_start(
                        out=slab[:, cb, :, :].rearrange("k r w -> k (r w)"),
                        in_=_dap(src, cb * P * srlen + i_lo * Wp,
                                 [[srlen, P], [1, rows_in * Wp]]))
                    rha = 1 if s_i == nstrip - 1 else 0
                    nc.scalar.activation(
                        out=slab[:, cb, :rows_in - rha, 0:Wi],
                        in_=slab[:, cb, :rows_in - rha, 0:Wi],
                        func=AF.Relu, bias=st_in[cb][:, 1:2],
                        scale=st_in[cb][:, 0:1])
                for m in range(cbo):
                    oslab = sb.tile([Mo, nr, Wq], BF16, name=f"osu{li}",
                                    tag="outslab")
                    nc.gpsimd.memset(oslab[:, :, Wo:Wq], 0.0)
                    nrec = 0
                    for a in range(2):
                        kys = [1] if a == 0 else [0, 2]
                        for b_ in range(2):
                            kxs = [1] if b_ == 0 else [0, 2]
                            for su in range(nsub):
                                yb = y0 + a + 2 * su * rsub
                                pt = ps.tile([Mo, rsub, Wi], F32,
                                             name=f"ptu{li}", tag="mm")
                                first = True
                                for cb in range(cbi):
                                    for ky in kys:
                                        i_first = (yb + 1 - ky) // 2
                                        for kx in kxs:
                                            j0 = (b_ + 1 - kx) // 2
                                            rhs = _ap(
                                                slab[:, 0, 0, 0],
                                                cb * rows_in * Wp
                                                + (i_first - i_lo) * Wp + j0,
                                                [[Wp, rsub], [1, Wi]])
                                            last = (cb == cbi - 1
                                                    and ky == kys[-1]
                                                    and kx == kxs[-1])
                                            nc.tensor.matmul(
                                                pt, wt[:, m, cb, ky, kx, :],
                                                rhs, start=first, stop=last)
                                            first = False
                                nc.vector.bn_stats(
                                    out=stt[:, m, s_i * 4 * nsub + nrec, :],
                                    in_=pt.rearrange("p a b -> p (a b)"))
                                nrec += 1
                                oap = _ap(oslab[:, 0, 0],
                                          (a + 2 * su * rsub) * Wq + b_,
                                          [[2 * Wq, rsub], [2, Wi]])
                                nc.scalar.activation(out=oap, in_=pt,
                                                     func=AF.Copy)
                    if s_i == 0:
                        nc.vector.tensor_copy(out=rfirst[:, m, :],
                                              in_=oslab[:, 0, 0:Wo])
                    nc.sync.dma_start(
                        out=_dap(dst, m * P * drlen + y0 * Wq,
                                 [[drlen, P], [1, nr * Wq]]),
                        in_=oslab.rearrange("p r w -> p (r w)"))
            mes = []
            for m in range(cbo):
                mv = stp.tile([P, 2], F32, name=f"mvu{li}", tag="mv")
                nc.vector.bn_aggr(out=mv, in_=stt[:, m, :, :])
                me = stp.tile([P, 2], F32, name=f"meu{li}m{m}", tag=f"mu{li}{m}")
                mv_to_me(mv, me)
                mes.append(me)
            cc_exchange(bname, mes,
                        [(rfirst[:, m, :], m, 4) for m in range(cbo)], P)
            st_outs = []
            for meg, cb in cc_stats(bname, cbo, P, f"u{li}"):
                st_outs.append(finalize_stats(meg, P, f"u{li}m{cb}"))
            # halo-below rows (slot 1 = peer's first row; masked by is_top)
            for m in range(cbo):
                hr = cc_halo(bname, m, P, Wo, 4, True, None)
                hw = sb.tile([P, Wq], BF16, name=f"hwu{li}", tag="halow")
                nc.gpsimd.memset(hw[:, Wo:Wq], 0.0)
                nc.scalar.activation(out=hr, in_=hr, func=AF.Relu,
                                     bias=st_outs[m][:, 1:2],
                                     scale=st_outs[m][:, 0:1])
                nc.vector.tensor_scalar(out=hw[:, 0:Wo], in0=hr,
                                        scalar1=maskt, scalar2=None,
                                        op0=ALU.mult)
                nc.gpsimd.dma_start(
                    out=_dap(dst, m * P * drlen + (dst.shape[2] - 1) * Wq,
                             [[drlen, P], [1, Wq]]),
                    in_=hw)
            return st_outs

        st_g0 = up_layer(0, h4, g0, wu[0], st_h4, "b5")
        st_g1 = up_layer(1, g0, g1, wu[1], st_g0, "b6")
        st_g2 = up_layer(2, g1, g2, wu[2], st_g1, "b7")

        # ================= u3: 128 -> 64, writes g3e/g3o ==================
        Wi3, Wo3 = 256, 512
        Wp3 = Wi3 + 1
        nstrip3, nr3, rsub3 = 64, 4, 2
        stt3 = stp.tile([64, 256, 6], F32, name="stu3", tag="stats")
        rfirst3 = nrm.tile([64, 3, 512], BF16, name="rf_u3")
        rlast3 = nrm.tile([64, 3, 512], BF16, name="rl_u3")
        wt3 = wsm.tile([P, 1, 1, 3, 3, 64], BF16, name="wtu3", tag="wshared",
                       bufs=1)
        nc.sync.dma_start(out=wt3, in_=wu[3][:, :, :, :, :, :])
        for s_i in range(nstrip3):
            y0 = s_i * nr3
            i_lo = y0 // 2
            rows_in = nr3 // 2 + 1
            slab = sb.tile([P, rows_in, Wp3], BF16, name="slu3", tag="inslab")
            nc.sync.dma_start(
                out=slab.rearrange("k r w -> k (r w)"),
                in_=_dap(g2, i_lo * Wp3, [[129 * 257, P], [1, rows_in * Wp3]]))
            rha = 1 if s_i == nstrip3 - 1 else 0
            nc.scalar.activation(out=slab[:, :rows_in - rha, 0:Wi3],
                                 in_=slab[:, :rows_in - rha, 0:Wi3],
                                 func=AF.Relu,
                                 bias=st_g2[0][:, 1:2], scale=st_g2[0][:, 0:1])
            osE = sb.tile([64, 2, 512], BF16, name="osE", tag="outslab")
            osO = sb.tile([64, 2, 512], BF16, name="osO", tag="outslab2")
            for a in range(2):
                kys = [1] if a == 0 else [0, 2]
                dst_t = osO if a == 0 else osE  # padded parity = (a+1)%2
                for b_ in range(2):
                    kxs = [1] if b_ == 0 else [0, 2]
                    pt = ps.tile([64, rsub3, Wi3], F32, name="ptu3", tag="mm")
                    first = True
                    for ky in kys:
                        i_first = (y0 + a + 1 - ky) // 2
                        for kx in kxs:
                            j0 = (b_ + 1 - kx) // 2
                            rhs = _ap(slab[:, 0, 0],
                                      (i_first - i_lo) * Wp3 + j0,
                                      [[Wp3, rsub3], [1, Wi3]])
                            last = (ky == kys[-1] and kx == kxs[-1])
                            nc.tensor.matmul(pt, wt3[:, 0, 0, ky, kx, :], rhs,
                                             start=first, stop=last)
                            first = False
                    nc.vector.bn_stats(out=stt3[:, s_i * 4 + a * 2 + b_, :],
                                       in_=pt.rearrange("p a b -> p (a b)"))
                    oap = _ap(dst_t[:, 0, 0], b_, [[512, rsub3], [2, Wi3]])
                    nc.scalar.activation(out=oap, in_=pt, func=AF.Copy)
            if s_i == 0:
                # first3 = padded rows 3,4,5 = osO[0], osE[0], osO[1]
                nc.vector.tensor_copy(out=rfirst3[:, 0, :], in_=osO[:, 0, :])
                nc.vector.tensor_copy(out=rfirst3[:, 1, :], in_=osE[:, 0, :])
                nc.vector.tensor_copy(out=rfirst3[:, 2, :], in_=osO[:, 1, :])
            if s_i == nstrip3 - 1:
                # last3 = padded rows 256,257,258 = osE[0], osO[1], osE[1]
                nc.vector.tensor_copy(out=rlast3[:, 0, :], in_=osE[:, 0, :])
                nc.vector.tensor_copy(out=rlast3[:, 1, :], in_=osO[:, 1, :])
                nc.vector.tensor_copy(out=rlast3[:, 2, :], in_=osE[:, 1, :])
            # store: osE rows k -> g3e idx y0/2+2+k; osO rows k -> g3o y0/2+1+k
            nc.sync.dma_start(
                out=_dap(g3e, (y0 // 2 + 2) * 512, [[131 * 512, 64], [1, 1024]]),
                in_=osE.rearrange("p r w -> p (r w)"))
            nc.sync.dma_start(
                out=_dap(g3o, (y0 // 2 + 1) * 512, [[131 * 512, 64], [1, 1024]]),
                in_=osO.rearrange("p r w -> p (r w)"))
        mv3 = stp.tile([64, 2], F32, name="mvu3", tag="mv")
        nc.vector.bn_aggr(out=mv3, in_=stt3)
        me3 = stp.tile([64, 2], F32, name="meu3", tag="meu3")
        mv_to_me(mv3, me3)
        cc_exchange("b8", [me3],
                    [(rfirst3.rearrange("p a w -> p (a w)"), 0, 4),
                     (rlast3.rearrange("p a w -> p (a w)"), 0, 4 + 3 * 512)], 64)
        st_g3 = finalize_stats(cc_stats("b8", 1, 64, "g3")[0][0], 64, "g3")
        # g3 padded boundary rows:
        #  above (padded 0,1,2): top = reflect own (6,5,4); bottom = peer last3
        #  below (padded 259,260,261): top = peer first3; bottom = reflect own
        #  (257,256,255)
        refl_src = [(g3e, 3), (g3o, 2), (g3e, 2)]        # for above
        refl_dst = [(g3e, 0), (g3o, 0), (g3e, 1)]
        refl_src_b = [(g3o, 128), (g3e, 128), (g3o, 127)]  # for below
        refl_dst_b = [(g3o, 129), (g3e, 130), (g3o, 130)]
        for k in range(3):
            for (srcs, dsts, peer_off, mask_peer, mask_own) in (
                    (refl_src, refl_dst, 4 + 3 * 512, maskb, maskt),
                    (refl_src_b, refl_dst_b, 4, maskt, maskb)):
                st_, si_ = srcs[k]
                dt_, di_ = dsts[k]
                own = sb.tile([64, 512], BF16, name="g3own", tag="halo2")
                nc.gpsimd.dma_start(
                    out=own, in_=_dap(st_, si_ * 512, [[131 * 512, 64], [1, 512]]))
                peer = cc_halo("b8", 0, 64, 512, peer_off + k * 512,
                               peer_off == 4, mask_peer)
                nc.vector.tensor_scalar(out=own, in0=own, scalar1=mask_own[:64],
                                        scalar2=None, op0=ALU.mult)
                nc.vector.tensor_add(out=own, in0=own, in1=peer)
                nc.gpsimd.dma_start(
                    out=_dap(dt_, di_ * 512, [[131 * 512, 64], [1, 512]]),
                    in_=own)

        nc.leave_named_scope("up", _sc, False)
        _sc = nc.enter_named_scope("final", False)[0]

        # ================= final conv 7x7, 64 -> 3, tanh ==================
        sF = st_g3
        wfAt = wsm.tile([P, 6, 126], BF16, name="wfAt", tag="wfA", bufs=1)
        nc.sync.dma_start(out=wfAt, in_=wfA[:, :, :])
        wfSt = wsm.tile([126, 7, 18], BF16, name="wfSt", tag="wfS", bufs=1)
        nc.sync.dma_start(out=wfSt, in_=wfS[:, :, :])
        bft = wsm.tile([18, 1], F32, name="bft", tag="bft", bufs=1)
        nc.sync.dma_start(out=bft, in_=bfv[:, :])
        edgebuf = stp.tile([126, 43, 14], BF16, name="edgebuf", tag="edge")

        for si, y0 in enumerate(Y0LIST):
            slab = sb.tile([P, 6, 512], BF16, name="slF", tag="inslab")
            nc.sync.dma_start(
                out=slab[0:64, :, :].rearrange("p r w -> p (r w)"),
                in_=_dap(g3e, (y0 // 2) * 512, [[131 * 512, 64], [1, 6 * 512]]))
            nc.sync.dma_start(
                out=slab[64:128, :, :].rearrange("p r w -> p (r w)"),
                in_=_dap(g3o, (y0 // 2) * 512, [[131 * 512, 64], [1, 6 * 512]]))
            nc.scalar.activation(out=slab[0:64], in_=slab[0:64], func=AF.Relu,
                                 bias=sF[:, 1:2], scale=sF[:, 0:1])
            nc.scalar.activation(out=slab[64:128], in_=slab[64:128],
                                 func=AF.Relu, bias=sF[:, 1:2],
                                 scale=sF[:, 0:1])
            pA = psf.tile([126, 512], F32, name="pA", tag="fa")
            for t in range(6):
                rhs = _ap(slab[:, 0, 0], t * 512, [[1, 512]])
                nc.tensor.matmul(pA, wfAt[:, t, :], rhs,
                                 start=(t == 0), stop=(t == 5))
            stg = sb.tile([126, 512], BF16, name="stg", tag="outslab")
            nc.scalar.activation(out=stg, in_=pA, func=AF.Copy)
            nc.vector.tensor_copy(out=edgebuf[:, si, 0:7], in_=stg[:, 0:7])
            nc.vector.tensor_copy(out=edgebuf[:, si, 7:14], in_=stg[:, 505:512])
            pB = psf.tile([18, 506], F32, name="pB", tag="fb", bufs=1)
            for dx in range(7):
                nc.tensor.matmul(pB, wfSt[:, dx, :], stg[:, dx:dx + 506],
                                 start=(dx == 0), stop=(dx == 6))
            ftile = sb.tile([18, 506], F32, name="ftile", tag="ftile")
            nc.scalar.activation(out=ftile, in_=pB, func=AF.Tanh,
                                 bias=bft, scale=1.0)
            nc.sync.dma_start(
                out=_dap(hfout, si * 506, [[43 * 506, 18], [1, 506]]),
                in_=ftile)

        # edge columns: out col x in {0,1,2, 509,510,511}
        pe_ = psf.tile([18, 6, 43], F32, name="pe", tag="fe", bufs=1)
        for xi, x in enumerate([0, 1, 2, 509, 510, 511]):
            for dx in range(7):
                n = x + dx - 3
                if n < 0:
                    n = -n
                elif n > 511:
                    n = 1022 - n
                col = n if n <= 6 else n - 498
                rhs = _ap(edgebuf[:, 0, 0], col, [[14, 43]])
                nc.tensor.matmul(pe_[:, xi, :], wfSt[:, dx, :], rhs,
                                 start=(dx == 0), stop=(dx == 6))
        eft = sb.tile([18, 6, 43], F32, name="eft", tag="ftile")
        nc.scalar.activation(out=eft, in_=pe_, func=AF.Tanh, bias=bft,
                             scale=1.0)
        nc.sync.dma_start(
            out=_dap(efout, 0, [[6 * 43, 18], [1, 6 * 43]]),
            in_=eft.rearrange("p a b -> p (a b)"))
        nc.leave_named_scope("final", _sc, False)

        if debug:
            for nm, tens in [("h0", h0), ("h1", h1), ("h2", h2), ("h3", h3),
                             ("h4", h4), ("g0", g0), ("g1", g1), ("g2", g2),
                             ("g3e", g3e), ("g3o", g3o)]:
                sh = tens.shape
                if len(sh) == 4:
                    nblk, npart, nfree = sh[0], sh[1], sh[2] * sh[3]
                else:
                    nblk, npart, nfree = 1, sh[0], sh[1] * sh[2]
                dbg = nc.dram_tensor("dbg_" + nm, [nblk, npart, nfree], BF16,
                                     kind="ExternalOutput")
                for blk in range(nblk):
                    nc.sync.dma_start(
                        out=_dap(dbg, blk * npart * nfree,
                                 [[nfree, npart], [1, nfree]]),
                        in_=_dap(tens, blk * npart * nfree,
                                 [[nfree, npart], [1, nfree]]))

    nc.finalize()
    return nc


# ----------------------------------------------------------------------------
# Host driver
# ----------------------------------------------------------------------------

def assemble_output(results, inst):
    """Build the full [B,3,H,W] output from per-core hfout/efout."""
    out = np.zeros((B, 3, H, W), np.float32)
    edge_cols = [0, 1, 2, 509, 510, 511]
    for b in range(B):
        hf = np.zeros((3, H, W), np.float32)
        for core, rowbase in ((b, 0), (b + 4, 256)):
            ft = np.asarray(results[core]["hfout"], np.float32)  # [18,43,506]
            ef = np.asarray(results[core]["efout"], np.float32)  # [18,6,43]
            for si, y0 in enumerate(Y0LIST):
                for r in range(6):
                    y = rowbase + y0 + r
                    hf[:, y, 3:509] = ft[r * 3:(r + 1) * 3, si, :]
                    for xi, x in enumerate(edge_cols):
                        hf[:, y, x] = ef[r * 3:(r + 1) * 3, xi, si]
        mask = (np.asarray(inst[b, 0]) == 1)
        cnt = float(mask.sum())
        mean = (hf * mask[None]).sum((1, 2)) / cnt
        out[b] = mean[:, None, None] * mask[None].astype(np.float32)
    return out


_CACHE = {}


def run(inputs, trace=False):
    if "nc" not in _CACHE:
        _CACHE["nc"] = build_kernel()
    nc = _CACHE["nc"]
    wblobs = prep_weights(inputs)
    x = np.asarray(inputs["x"], np.float32)
    in_maps = [prep_core_inputs(x[c % B], c >= B, wblobs) for c in range(8)]
    res = run_bass_kernel_spmd(nc, in_maps, core_ids=list(range(8)),
                               trace=trace)
    return res


def kernel(**inputs):
    res = run(inputs)
    return assemble_output(res.results, np.asarray(inputs["inst"]))


# revision 14
# speedup vs baseline: 1.0707x; 1.0707x over previous
# Trainium2 Bass kernel for nn_Encoder_81509889343552.
# Encoder-decoder CNN (7x7 conv -> 4 stride-2 convs -> 4 stride-2 convT -> 7x7
# conv + InstanceNorm/ReLU, tanh) followed by a masked segment mean.
#
# Sharding: H-split SPMD over 8 cores. Core c (0-3) computes the TOP half of
# image c; core c+4 the BOTTOM half. Per layer boundary one pairwise AllGather
# exchanges (a) per-channel partial InstanceNorm stats [mean, E[x^2]] (exact
# global stats = average of the two halves) and (b) the raw boundary row(s)
# needed as conv halos. The program is identical on all cores; top/bottom
# asymmetry is data-driven via 0/1 mask inputs.
#
# All conv matmuls run in bf16 (full PE rate), accumulating fp32 in PSUM.
# Activations are stored in HBM as bf16 in consumer-friendly layouts with
# baked-in zero gutter columns so every DMA is one contiguous chunk per
# partition. Weights are pre-transposed on the host. The final 7x7 conv
# computes output columns 3..508 on device (no column reflect needed); the 6
# reflected edge columns are produced by a small batched matmul pass over
# saved stage-A partials. The masked segment mean is finished on the host
# from the per-core tanh outputs.
import sys

sys.path.insert(0, "/opt/trn_rl_repo")

import contextlib

import numpy as np
import ml_dtypes

import concourse.bass as bass
import concourse.bacc as bacc
import concourse.tile as tile
from concourse import mybir
from concourse.bass_utils import run_bass_kernel_spmd

F32 = mybir.dt.float32
BF16 = mybir.dt.bfloat16
AF = mybir.ActivationFunctionType
ALU = mybir.AluOpType
BF = ml_dtypes.bfloat16

B, H, W = 4, 512, 512
EPS = 1e-5
P = 128
OWN = 256  # h0 rows owned per core
PAIRS = [[0, 4], [1, 5], [2, 6], [3, 7]]

# down layers: (Cin, Cout, Wi, own_out, nr, nrc)
DCFG = [(64, 128, 512, 128, 8, 2),
        (128, 256, 256, 64, 8, 4),
        (256, 512, 128, 32, 8, 8),
        (512, 1024, 64, 16, 16, 16)]
# up layers 0-2: (Cin, Cout, Wi, own_out, nr, rsub)
UCFG = [(1024, 512, 32, 32, 32, 16),
        (512, 256, 64, 64, 16, 8),
        (256, 128, 128, 128, 8, 4)]
# u3: 128 -> 64, Wi=256, own_out=256, nr=4, rsub=2

Y0LIST = list(range(0, 250, 6)) + [250]  # 43 final-conv strips per core


def _ap(base, extra_off, dims):
    return bass.AP(tensor=base.tensor, offset=base.offset + extra_off,
                   ap=[list(base.ap[0])] + [list(d) for d in dims])


def _dap(handle, off, dims):
    return bass.AP(tensor=handle, offset=off, ap=[list(d) for d in dims])


# ----------------------------------------------------------------------------
# Host-side weight preprocessing (all blobs bf16, K-partition-major)
# ----------------------------------------------------------------------------

def prep_weights(inp):
    w = {}
    # L1 7x7: K=96: p = ci*32 + dy*4 + l (dy 0..7, l 0..3); matmul d (0..1)
    # covers kx = 4d + l; M = r*64 + co (r 0..1).
    w0 = np.asarray(inp["w0"], np.float32)  # [64, 3, 7, 7]
    w1 = np.zeros((96, 2, 128), np.float32)
    for ci in range(3):
        for dy in range(8):
            for l in range(4):
                p = ci * 32 + dy * 4 + l
                for d in range(2):
                    kx = 4 * d + l
                    if kx > 6:
                        continue
                    for r in range(2):
                        ky = dy - r
                        if 0 <= ky <= 6:
                            w1[p, d, r * 64:(r + 1) * 64] = w0[:, ci, ky, kx]
    w["w1"] = w1.astype(BF)

    # down convs: [K, cbo, cbi, 3, 3, 128]
    c = 64
    for i in range(4):
        dw = np.asarray(inp[f"dw{i}"], np.float32)  # [2c, c, 3, 3]
        cbo, cbi, K = (2 * c) // P, max(c // P, 1), min(c, P)
        blob = np.zeros((K, cbo, cbi, 3, 3, P), np.float32)
        for m in range(cbo):
            for cb in range(cbi):
                blob[:, m, cb] = dw[m * P:(m + 1) * P,
                                    cb * K:(cb + 1) * K].transpose(1, 2, 3, 0)
        w[f"wd{i}"] = blob.astype(BF)
        c *= 2

    # up convs (torch convT layout uw [Cin, Cout, 3, 3]): [128, cbo, cbi, 3, 3, Mo]
    for i in range(4):
        uw = np.asarray(inp[f"uw{i}"], np.float32)
        Cin_, Cout_ = uw.shape[0], uw.shape[1]
        cbi, cbo, Mo = Cin_ // P, max(Cout_ // P, 1), min(Cout_, P)
        blob = np.zeros((P, cbo, cbi, 3, 3, Mo), np.float32)
        for m in range(cbo):
            for cb in range(cbi):
                blob[:, m, cb] = uw[cb * P:(cb + 1) * P,
                                    m * Mo:(m + 1) * Mo].transpose(0, 2, 3, 1)
        w[f"wu{i}"] = blob.astype(BF)
        c //= 2

    # final 7x7 stage A: K = j*64+ch, M = r*21 + dx*3 + co; round t reads
    # padded rows y0+2t+j => ky = 2t + j - r.
    wf = np.asarray(inp["wf"], np.float32)  # [3, 64, 7, 7]
    wfA = np.zeros((128, 6, 126), np.float32)
    for t in range(6):
        for j in range(2):
            for r in range(6):
                ky = 2 * t + j - r
                if 0 <= ky <= 6:
                    for dx in range(7):
                        for co in range(3):
                            wfA[j * 64:(j + 1) * 64, t, r * 21 + dx * 3 + co] = \
                                wf[co, :, ky, dx]
    # stage B gather: [126, 7, 18]
    wfS = np.zeros((126, 7, 18), np.float32)
    for dx in range(7):
        for r in range(6):
            for co in range(3):
                wfS[r * 21 + dx * 3 + co, dx, r * 3 + co] = 1.0
    w["wfA"] = wfA.astype(BF)
    w["wfS"] = wfS.astype(BF)
    bf = np.asarray(inp["bf"], np.float32)
    w["bfv"] = np.tile(bf, 6).reshape(18, 1).astype(np.float32)

    # L1 stats r-fold: average partition pairs (r*64+co) -> co
    wfold = np.zeros((128, 64), np.float32)
    for r in range(2):
        for co in range(64):
            wfold[r * 64 + co, co] = 0.5
    w["wfold"] = wfold.astype(BF)
    return w


def prep_core_inputs(x_img, is_bottom, wblobs):
    # xrep: [96, 128, 518] bf16 per core; row-pair index s covers h0 rows
    # (2s, 2s+1) in core-local coords; p = ci*32 + dy*4 + l reads
    # xpad[ci, 2s + base + dy, l : l+515...].
    xpad = np.pad(np.asarray(x_img, np.float32), ((0, 0), (3, 3), (3, 3)),
                  mode="reflect")  # [3, 518, 518]
    base = 256 if is_bottom else 0
    xrep = np.zeros((96, 128, 518), np.float32)
    for ci in range(3):
        for dy in range(8):
            for l in range(4):
                p = ci * 32 + dy * 4 + l
                rows = xpad[ci, base + dy: base + dy + 256: 2, l:]
                xrep[p, :, :rows.shape[1]] = rows
    m = {"xrep": xrep.astype(BF),
         "maskb": np.full((P, 1), float(is_bottom), np.float32),
         "maskt": np.full((P, 1), float(not is_bottom), np.float32)}
    m.update(wblobs)
    return m


# ----------------------------------------------------------------------------
# Device kernel
# ----------------------------------------------------------------------------

def build_kernel(debug=False):
    nc = bacc.Bacc(None, target_bir_lowering=False)

    xrep = nc.dram_tensor("xrep", [96, 128, 518], BF16, kind="ExternalInput")
    maskb_h = nc.dram_tensor("maskb", [P, 1], F32, kind="ExternalInput")
    maskt_h = nc.dram_tensor("maskt", [P, 1], F32, kind="ExternalInput")
    w1 = nc.dram_tensor("w1", [96, 2, 128], BF16, kind="ExternalInput")
    wd, wu = [], []
    c = 64
    for i in range(4):
        cbo, cbi, K = (2 * c) // P, max(c // P, 1), min(c, P)
        wd.append(nc.dram_tensor(f"wd{i}", [K, cbo, cbi, 3, 3, P], BF16,
                                 kind="ExternalInput"))
        c *= 2
    for i in range(4):
        cbi, cbo, Mo = c // P, max((c // 2) // P, 1), min(c // 2, P)
        wu.append(nc.dram_tensor(f"wu{i}", [P, cbo, cbi, 3, 3, Mo], BF16,
                                 kind="ExternalInput"))
        c //= 2
    wfA = nc.dram_tensor("wfA", [128, 6, 126], BF16, kind="ExternalInput")
    wfS = nc.dram_tensor("wfS", [126, 7, 18], BF16, kind="ExternalInput")
    bfv = nc.dram_tensor("bfv", [18, 1], F32, kind="ExternalInput")
    wfold = nc.dram_tensor("wfold", [128, 64], BF16, kind="ExternalInput")

    # activations (bf16). Down-consumed: [cb][128][slots][W+1], col 0 = zero
    # gutter, slot 0 = halo-above. Up-consumed: data cols 0..W-1, col W = 0,
    # last slot = halo-below.
    h0 = nc.dram_tensor("h0", [64, 257, 513], BF16)
    h1 = nc.dram_tensor("h1", [1, P, 129, 257], BF16)
    h2 = nc.dram_tensor("h2", [2, P, 65, 129], BF16)
    h3 = nc.dram_tensor("h3", [4, P, 33, 65], BF16)
    h4 = nc.dram_tensor("h4", [8, P, 17, 33], BF16)
    g0 = nc.dram_tensor("g0", [4, P, 33, 65], BF16)
    g1 = nc.dram_tensor("g1", [2, P, 65, 129], BF16)
    g2 = nc.dram_tensor("g2", [1, P, 129, 257], BF16)
    # g3 split by padded-row parity, unpadded width: rows = padded 0..261
    g3e = nc.dram_tensor("g3e", [64, 131, 512], BF16)  # even padded rows
    g3o = nc.dram_tensor("g3o", [64, 131, 512], BF16)  # odd padded rows
    dsts_down = [h1, h2, h3, h4]
    srcs_up = [h4, g0, g1, g2]

    # collective buffers (bf16 payload; stats are fp32 bitcast into 4 cols)
    ccin, ccout = {}, {}
    for name, cbo, Wc, nrows in [("b0", 1, 512, 1), ("b1", 1, 256, 1),
                                 ("b2", 2, 128, 1), ("b3", 4, 64, 1),
                                 ("b4", 8, 32, 1), ("b5", 4, 64, 1),
                                 ("b6", 2, 128, 1), ("b7", 1, 256, 1),
                                 ("b8", 1, 512, 3)]:
        F = 4 + nrows * Wc * (2 if name == "b8" else 1)
        ccin[name] = nc.dram_tensor(f"ccin_{name}", [cbo, P, F], BF16)
        ccout[name] = nc.dram_tensor(f"ccout_{name}", [2, cbo, P, F], BF16)

    hfout = nc.dram_tensor("hfout", [18, 43, 506], F32, kind="ExternalOutput")
    efout = nc.dram_tensor("efout", [18, 6, 43], F32, kind="ExternalOutput")

    with tile.TileContext(nc) as tc, contextlib.ExitStack() as ctx:
        sb = ctx.enter_context(tc.tile_pool(name="sb", bufs=3))
        wsm = ctx.enter_context(tc.tile_pool(name="wsm", bufs=2))
        nrm = ctx.enter_context(tc.tile_pool(name="nrm", bufs=1))
        stp = ctx.enter_context(tc.tile_pool(name="stp", bufs=1))
        ps = ctx.enter_context(tc.tile_pool(name="ps", bufs=3, space="PSUM"))
        psf = ctx.enter_context(tc.tile_pool(name="psf", bufs=2, space="PSUM"))

        eps_t = nrm.tile([P, 1], F32, name="eps_t")
        nc.vector.memset(eps_t, EPS)
        maskb = nrm.tile([P, 1], F32, name="maskb_t")
        nc.sync.dma_start(out=maskb, in_=maskb_h[:, :])
        maskt = nrm.tile([P, 1], F32, name="maskt_t")
        nc.sync.dma_start(out=maskt, in_=maskt_h[:, :])

        def finalize_stats_batch(meg, npart, cbo, name):
            """meg [npart, cbo, 2] = (mean, E[x^2]) -> [(scale, bias)] APs."""
            s_t = nrm.tile([npart, cbo, 2], F32, name=f"st_{name}",
                           tag=f"st_{name}")
            var = stp.tile([npart, cbo, 2], F32, name=f"var_{name}",
                           tag="stmp2")
            nc.vector.tensor_mul(out=var[:, :, 0:1], in0=meg[:, :, 0:1],
                                 in1=meg[:, :, 0:1])
            nc.vector.tensor_sub(out=var[:, :, 1:2], in0=meg[:, :, 1:2],
                                 in1=var[:, :, 0:1])
            tmp = stp.tile([npart, cbo, 1], F32, name=f"tmp_{name}",
                           tag="stmp")
            nc.scalar.activation(out=tmp, in_=var[:, :, 1:2], func=AF.Sqrt,
                                 bias=eps_t[:npart], scale=1.0)
            nc.vector.reciprocal(out=s_t[:, :, 0:1], in_=tmp)
            nc.vector.tensor_mul(out=s_t[:, :, 1:2], in0=meg[:, :, 0:1],
                                 in1=s_t[:, :, 0:1])
            nc.vector.tensor_scalar(out=s_t[:, :, 1:2], in0=s_t[:, :, 1:2],
                                    scalar1=-1.0, scalar2=None, op0=ALU.mult)
            return s_t, [(s_t[:, cb, 0:1], s_t[:, cb, 1:2])
                         for cb in range(cbo)]

        def mv_to_me(mv_ap, me_all, m):
            """bn_aggr [mean, var] -> me_all[:, m, :] = [mean, E[x^2]]."""
            nc.vector.tensor_copy(out=me_all[:, m, 0:1], in_=mv_ap[:, 0:1])
            nc.vector.tensor_scalar(out=me_all[:, m, 1:2], in0=mv_ap[:, 0:1],
                                    scalar1=mv_ap[:, 0:1], scalar2=None,
                                    op0=ALU.mult)
            nc.vector.tensor_add(out=me_all[:, m, 1:2], in0=me_all[:, m, 1:2],
                                 in1=mv_ap[:, 1:2])

        def cc_exchange(name, me_all, row_tiles, npart):
            """Write stats+rows to ccin, AllGather.

            me_all: [npart, cbo, 2] fp32 tile. row_tiles: list of
            (bf16 tile [npart, cbo, Wrow] or [npart, Wrow], col_offset).
            """
            ci, co_ = ccin[name], ccout[name]
            F = ci.shape[2]
            cbo = me_all.shape[1]
            nc.gpsimd.dma_start(
                out=_dap(ci, 0, [[F, npart], [P * F, cbo], [1, 4]]),
                in_=me_all.bitcast(BF16).rearrange("p c f -> p (c f)"))
            for (rt, coff) in row_tiles:
                np_r = rt.shape[0]
                if len(rt.shape) == 3:
                    wrow = rt.shape[2]
                    nbl = rt.shape[1]
                    rt = rt.rearrange("p c w -> p (c w)")
                else:
                    wrow, nbl = rt.shape[1], 1
                nc.gpsimd.dma_start(
                    out=_dap(ci, coff,
                             [[F, np_r], [P * F, nbl], [1, wrow]]),
                    in_=rt)
            nc.gpsimd.collective_compute(
                "AllGather", mybir.AluOpType.bypass, replica_groups=PAIRS,
                ins=[ci[:, :, :]], outs=[co_[:, :, :, :]])
            return co_

        def cc_stats(name, cbo, npart, sname):
            """Read gathered stats, average halves -> meg [npart, cbo, 2]."""
            co_ = ccout[name]
            F = co_.shape[3]
            sa = stp.tile([npart, 2, cbo, 4], BF16, name=f"sa_{sname}",
                          tag="ccst")
            nc.gpsimd.dma_start(
                out=sa.rearrange("p s c f -> p (s c f)"),
                in_=_dap(co_, 0, [[F, npart], [cbo * P * F, 2],
                                  [P * F, cbo], [1, 4]]))
            saf = sa.bitcast(F32)  # [npart, 2, cbo, 2]
            meg = stp.tile([npart, cbo, 2], F32, name=f"meg_{sname}",
                           tag="ccme")
            nc.vector.tensor_add(out=meg, in0=saf[:, 0], in1=saf[:, 1])
            nc.vector.tensor_scalar(out=meg, in0=meg, scalar1=0.5,
                                    scalar2=None, op0=ALU.mult)
            return meg

        def cc_halo(name, cb, npart, wrow, coff, slot_hi, mask, nbl=1):
            """Halo rows from gathered slot -> bf16 tile [npart, nbl, wrow]."""
            co_ = ccout[name]
            F = co_.shape[3]
            cbo = co_.shape[1]
            src_off = ((cbo if slot_hi else 0) + cb) * P * F + coff
            hrow = sb.tile([npart, nbl, wrow], BF16, name=f"hr_{name}",
                           tag="halo")
            nc.gpsimd.dma_start(
                out=hrow.rearrange("p c w -> p (c w)"),
                in_=_dap(co_, src_off,
                         [[F, npart], [P * F, nbl], [1, wrow]]))
            if mask is not None:
                nc.vector.tensor_scalar(out=hrow, in0=hrow,
                                        scalar1=mask[:npart],
                                        scalar2=None, op0=ALU.mult)
            return hrow

        # ================= L1: 7x7 conv, 3 -> 64 =========================
        _sc = nc.enter_named_scope("L1", False)[0]
        w1t = wsm.tile([96, 2, 128], BF16, name="w1t", tag="w1", bufs=1)
        nc.sync.dma_start(out=w1t, in_=w1[:, :, :])
        wft = wsm.tile([P, 64], BF16, name="wft", tag="wfold", bufs=1)
        nc.sync.dma_start(out=wft, in_=wfold[:, :])

        st1 = stp.tile([P, 128, 6], F32, name="st1", tag="stats")
        rl_h0 = nrm.tile([64, 512], BF16, name="rl_h0")  # last own h0 row
        NS1 = 32
        for s_i in range(NS1):
            slab = sb.tile([96, 4, 518], BF16, name="slab1", tag="inslab")
            nc.sync.dma_start(out=slab,
                              in_=_dap(xrep, s_i * 4 * 518,
                                       [[128 * 518, 96], [1, 4 * 518]]))
            oslab = sb.tile([64, 8, 513], BF16, name="oslab1", tag="outslab")
            nc.gpsimd.memset(oslab[:, :, 0:1], 0.0)
            for k in range(4):
                pt = ps.tile([P, 512], F32, name="pt1", tag="mm")
                for d in range(2):
                    rhs = _ap(slab[:, 0, 0], k * 518 + 4 * d, [[1, 512]])
                    nc.tensor.matmul(pt, w1t[:, d, :], rhs,
                                     start=(d == 0), stop=(d == 1))
                nc.vector.bn_stats(out=st1[:, s_i * 4 + k, :], in_=pt)
                nc.scalar.activation(out=oslab[:, 2 * k, 1:513],
                                     in_=pt[0:64, :], func=AF.Copy)
                nc.scalar.activation(out=oslab[:, 2 * k + 1, 1:513],
                                     in_=pt[64:128, :], func=AF.Copy)
            if s_i == NS1 - 1:
                nc.vector.tensor_copy(out=rl_h0, in_=oslab[:, 7, 1:513])
            nc.sync.dma_start(
                out=_dap(h0, (1 + s_i * 8) * 513, [[257 * 513, 64], [1, 8 * 513]]),
                in_=oslab)
        mv1 = stp.tile([P, 2], F32, name="mv1", tag="mv")
        nc.vector.bn_aggr(out=mv1, in_=st1)
        me1 = stp.tile([P, 1, 2], F32, name="me1", tag="me")
        mv_to_me(mv1, me1, 0)
        cc_exchange("b0", me1, [(rl_h0, 4)], P)
        # stats: average halves -> fold r-pairs via matmul -> scale/bias
        meg0 = cc_stats("b0", 1, P, "h0")
        megb = stp.tile([P, 2], BF16, name="megb", tag="megb")
        nc.vector.tensor_copy(out=megb, in_=meg0[:, 0, :])
        pm = ps.tile([64, 2], F32, name="pm", tag="mini", bufs=1)
        nc.tensor.matmul(pm, wft, megb, start=True, stop=True)
        me0g = stp.tile([64, 1, 2], F32, name="me0g", tag="mvg")
        nc.scalar.activation(out=me0g[:, 0, :], in_=pm, func=AF.Copy)
        _, st_h0 = finalize_stats_batch(me0g, 64, 1, "h0")
        # halo row -> h0 slot 0 (top: zeros = zero pad); post-norm + masked
        hrow = cc_halo("b0", 0, 64, 512, 4, False, None)
        hwr = sb.tile([64, 513], BF16, name="hwr0", tag="halow")
        nc.gpsimd.memset(hwr[:, 0:1], 0.0)
        nc.scalar.activation(out=hrow, in_=hrow, func=AF.Relu,
                             bias=st_h0[0][1], scale=st_h0[0][0])
        nc.vector.tensor_scalar(out=hwr[:, 1:513], in0=hrow[:, 0, :],
                                scalar1=maskb[:64], scalar2=None, op0=ALU.mult)
        nc.gpsimd.dma_start(out=_dap(h0, 0, [[257 * 513, 64], [1, 513]]), in_=hwr)

        nc.leave_named_scope("L1", _sc, False)
        _sc = nc.enter_named_scope("down", False)[0]

        # ================= down convs =====================================
        def down_layer(li, src, dst, wsrc, st_in, bname):
            Cin, Cout, Wi, own_out, nr, nrc = DCFG[li]
            up_dst = (li == 3)  # h4 is consumed by an up-conv layer
            Wo = Wi // 2
            cbi, cbo, K = max(Cin // P, 1), Cout // P, min(Cin, P)
            Wp, Wq = Wi + 1, Wo + 1
            nstrip, nchunk = own_out // nr, nr // nrc
            rows_in = 2 * nr + 1
            stt = stp.tile([P, cbo, nstrip * nchunk, 6], F32,
                           name=f"std{li}", tag="stats")
            rlast = nrm.tile([P, cbo, Wo], BF16, name=f"rl_d{li}")
            gcol = slice(Wo, Wq) if up_dst else slice(0, 1)
            dcol = slice(0, Wo) if up_dst else slice(1, Wq)
            # weights resident for the whole layer
            wt = wsm.tile([K, cbo, cbi, 3, 3, P], BF16, name=f"wtd{li}",
                          tag="wshared", bufs=1)
            nc.sync.dma_start(out=wt, in_=wsrc[:, :, :, :, :, :])
            for s_i in range(nstrip):
                slab = sb.tile([K, cbi, rows_in, Wp], BF16,
                               name=f"sld{li}", tag="inslab")
                for cb in range(cbi):
                    if src is h0:
                        sap = _dap(h0, (2 * s_i * nr) * 513,
                                   [[257 * 513, 64], [1, rows_in * 513]])
                    else:
                        sap = _dap(src, cb * P * src.shape[2] * src.shape[3]
                                   + (2 * s_i * nr) * Wp,
                                   [[src.shape[2] * src.shape[3], P],
                                    [1, rows_in * Wp]])
                    nc.sync.dma_start(
                        out=slab[:, cb, :, :].rearrange("k r w -> k (r w)"),
                        in_=sap)
                    r0a = 1 if s_i == 0 else 0  # halo row is already post-norm
                    nc.scalar.activation(
                        out=slab[:, cb, r0a:, 1:Wp],
                        in_=slab[:, cb, r0a:, 1:Wp],
                        func=AF.Relu, bias=st_in[cb][1],
                        scale=st_in[cb][0])
                oslab = sb.tile([P, cbo, nr, Wq], BF16, name=f"osd{li}",
                                tag="outslab")
                nc.gpsimd.memset(oslab[:, :, :, gcol], 0.0)
                for m in range(cbo):
                    for chk in range(nchunk):
                        pt = ps.tile([P, nrc, Wo], F32, name=f"ptd{li}",
                                     tag="mm")
                        first = True
                        for cb in range(cbi):
                            for dy in range(3):
                                for dx in range(3):
                                    row0 = 2 * chk * nrc + dy
                                    rhs = _ap(slab[:, 0, 0, 0],
                                              cb * rows_in * Wp + row0 * Wp + dx,
                                              [[2 * Wp, nrc], [2, Wo]])
                                    last = (cb == cbi - 1 and dy == 2
                                            and dx == 2)
                                    nc.tensor.matmul(
                                        pt, wt[:, m, cb, dy, dx, :], rhs,
                                        start=first, stop=last)
                                    first = False
                        nc.vector.bn_stats(
                            out=stt[:, m, s_i * nchunk + chk, :],
                            in_=pt.rearrange("p a b -> p (a b)"))
                        nc.scalar.activation(
                            out=oslab[:, m, chk * nrc:(chk + 1) * nrc, dcol],
                            in_=pt, func=AF.Copy)
                if up_dst and s_i == 0:
                    nc.vector.tensor_copy(out=rlast,
                                          in_=oslab[:, :, 0, dcol])
                if not up_dst and s_i == nstrip - 1:
                    nc.vector.tensor_copy(out=rlast,
                                          in_=oslab[:, :, nr - 1, dcol])
                row_base = s_i * nr if up_dst else 1 + s_i * nr
                for m in range(cbo):
                    nc.sync.dma_start(
                        out=_dap(dst, m * P * dst.shape[2] * dst.shape[3]
                                 + row_base * Wq,
                                 [[dst.shape[2] * dst.shape[3], P],
                                  [1, nr * Wq]]),
                        in_=oslab[:, m, :, :].rearrange("p r w -> p (r w)"))
            # stats + boundary exchange
            me_all = stp.tile([P, cbo, 2], F32, name=f"med{li}",
                              tag=f"me{li}")
            for m in range(cbo):
                mv = stp.tile([P, 2], F32, name=f"mvd{li}", tag="mv")
                nc.vector.bn_aggr(out=mv, in_=stt[:, m, :, :])
                mv_to_me(mv, me_all, m)
            cc_exchange(bname, me_all, [(rlast, 4)], P)
            meg = cc_stats(bname, cbo, P, f"d{li}")
            _, st_outs = finalize_stats_batch(meg, P, cbo, f"d{li}")
            # halo row (down-style: peer last row -> slot 0, masked maskb;
            # up-style dst: peer first row -> last slot, masked maskt)
            hr_all = cc_halo(bname, 0, P, Wo, 4, up_dst, None, nbl=cbo)
            hw_all = sb.tile([P, cbo, Wq], BF16, name=f"hwd{li}", tag="halow")
            nc.gpsimd.memset(hw_all[:, :, gcol], 0.0)
            for m in range(cbo):
                nc.scalar.activation(out=hw_all[:, m, dcol],
                                     in_=hr_all[:, m, :], func=AF.Relu,
                                     bias=st_outs[m][1],
                                     scale=st_outs[m][0])
            nc.vector.tensor_scalar(out=hw_all[:, :, dcol],
                                    in0=hw_all[:, :, dcol],
                                    scalar1=maskt if up_dst else maskb,
                                    scalar2=None, op0=ALU.mult)
            halo_row = dst.shape[2] - 1 if up_dst else 0
            S_ = dst.shape[2] * dst.shape[3]
            nc.gpsimd.dma_start(
                out=_dap(dst, halo_row * Wq,
                         [[S_, P], [P * S_, cbo], [1, Wq]]),
                in_=hw_all.rearrange("p c w -> p (c w)"))
            return st_outs

        st_h1 = down_layer(0, h0, h1, wd[0], st_h0, "b1")
        st_h2 = down_layer(1, h1, h2, wd[1], st_h1, "b2")
        st_h3 = down_layer(2, h2, h3, wd[2], st_h2, "b3")
        st_h4 = down_layer(3, h3, h4, wd[3], st_h3, "b4")

        nc.leave_named_scope("down", _sc, False)
        _sc = nc.enter_named_scope("up", False)[0]

        # ================= up convs 0-2 ===================================
        def up_layer(li, src, dst, wsrc, st_in, bname):
            Cin, Cout, Wi, own_out, nr, rsub = UCFG[li]
            Wo = 2 * Wi
            cbi, cbo, Mo = Cin // P, Cout // P, P
            Wp, Wq = Wi + 1, Wo + 1
            nstrip = own_out // nr
            n_cr = nr // 2
            nsub = n_cr // rsub
            srlen = src.shape[2] * src.shape[3]
            drlen = dst.shape[2] * dst.shape[3]
            stt = stp.tile([P, cbo, nstrip * 4 * nsub, 6], F32,
                           name=f"stu{li}", tag="stats")
            rfirst = nrm.tile([P, cbo, Wo], BF16, name=f"rf_u{li}")
            wt = wsm.tile([P, cbo, cbi, 3, 3, Mo], BF16, name=f"wtu{li}",
                          tag="wshared", bufs=1)
            nc.sync.dma_start(out=wt, in_=wsrc[:, :, :, :, :, :])
            for s_i in range(nstrip):
                y0 = s_i * nr
                i_lo = y0 // 2
                rows_in = nr // 2 + 1
                slab = sb.tile([P, cbi, rows_in, Wp], BF16,
                               name=f"slu{li}", tag="inslab")
                for cb in range(cbi):
                    nc.sync.dma_start(
                        out=slab[:, cb, :, :].rearrange("k r w -> k (r w)"),
                        in_=_dap(src, cb * P * srlen + i_lo * Wp,
                                 [[srlen, P], [1, rows_in * Wp]]))
                    rha = 1 if s_i == nstrip - 1 else 0
                    nc.scalar.activation(
                        out=slab[:, cb, :rows_in - rha, 0:Wi],
                        in_=slab[:, cb, :rows_in - rha, 0:Wi],
                        func=AF.Relu, bias=st_in[cb][1],
                        scale=st_in[cb][0])
                for m in range(cbo):
                    oslab = sb.tile([Mo, nr, Wq], BF16, name=f"osu{li}",
                                    tag="outslab")
                    nc.gpsimd.memset(oslab[:, :, Wo:Wq], 0.0)
                    nrec = 0
                    for a in range(2):
                        kys = [1] if a == 0 else [0, 2]
                        for b_ in range(2):
                            kxs = [1] if b_ == 0 else [0, 2]
                            for su in range(nsub):
                                yb = y0 + a + 2 * su * rsub
                                pt = ps.tile([Mo, rsub, Wi], F32,
                                             name=f"ptu{li}", tag="mm")
                                first = True
                                for cb in range(cbi):
                                    for ky in kys:
                                        i_first = (yb + 1 - ky) // 2
                                        for kx in kxs:
                                            j0 = (b_ + 1 - kx) // 2
                                            rhs = _ap(
                                                slab[:, 0, 0, 0],
                                                cb * rows_in * Wp
                                                + (i_first - i_lo) * Wp + j0,
                                                [[Wp, rsub], [1, Wi]])
                                            last = (cb == cbi - 1
                                                    and ky == kys[-1]
                                                    and kx == kxs[-1])
                                            nc.tensor.matmul(
                                                pt, wt[:, m, cb, ky, kx, :],
                                                rhs, start=first, stop=last)
                                            first = False
                                nc.vector.bn_stats(
                                    out=stt[:, m, s_i * 4 * nsub + nrec, :],
                                    in_=pt.rearrange("p a b -> p (a b)"))
                                nrec += 1
                                oap = _ap(oslab[:, 0, 0],
                                          (a + 2 * su * rsub) * Wq + b_,
                                          [[2 * Wq, rsub], [2, Wi]])
                                nc.scalar.activation(out=oap, in_=pt,
                                                     func=AF.Copy)
                    if s_i == 0:
                        nc.vector.tensor_copy(out=rfirst[:, m, :],
                                              in_=oslab[:, 0, 0:Wo])
                    nc.sync.dma_start(
                        out=_dap(dst, m * P * drlen + y0 * Wq,
                                 [[drlen, P], [1, nr * Wq]]),
                        in_=oslab.rearrange("p r w -> p (r w)"))
            me_all = stp.tile([P, cbo, 2], F32, name=f"meu{li}",
                              tag=f"mu{li}")
            for m in range(cbo):
                mv = stp.tile([P, 2], F32, name=f"mvu{li}", tag="mv")
                nc.vector.bn_aggr(out=mv, in_=stt[:, m, :, :])
                mv_to_me(mv, me_all, m)
            cc_exchange(bname, me_all, [(rfirst, 4)], P)
            meg = cc_stats(bname, cbo, P, f"u{li}")
            _, st_outs = finalize_stats_batch(meg, P, cbo, f"u{li}")
            # halo-below rows (slot 1 = peer's first row; masked by is_top)
            hr_all = cc_halo(bname, 0, P, Wo, 4, True, None, nbl=cbo)
            hw_all = sb.tile([P, cbo, Wq], BF16, name=f"hwu{li}", tag="halow")
            nc.gpsimd.memset(hw_all[:, :, Wo:Wq], 0.0)
            for m in range(cbo):
                nc.scalar.activation(out=hw_all[:, m, 0:Wo],
                                     in_=hr_all[:, m, :], func=AF.Relu,
                                     bias=st_outs[m][1],
                                     scale=st_outs[m][0])
            nc.vector.tensor_scalar(out=hw_all[:, :, 0:Wo],
                                    in0=hw_all[:, :, 0:Wo],
                                    scalar1=maskt, scalar2=None, op0=ALU.mult)
            nc.gpsimd.dma_start(
                out=_dap(dst, (dst.shape[2] - 1) * Wq,
                         [[drlen, P], [P * drlen, cbo], [1, Wq]]),
                in_=hw_all.rearrange("p c w -> p (c w)"))
            return st_outs

        st_g0 = up_layer(0, h4, g0, wu[0], st_h4, "b5")
        st_g1 = up_layer(1, g0, g1, wu[1], st_g0, "b6")
        st_g2 = up_layer(2, g1, g2, wu[2], st_g1, "b7")

        # ================= u3: 128 -> 64, writes g3e/g3o ==================
        Wi3, Wo3 = 256, 512
        Wp3 = Wi3 + 1
        nstrip3, nr3, rsub3 = 64, 4, 2
        stt3 = stp.tile([64, 256, 6], F32, name="stu3", tag="stats")
        rfirst3 = nrm.tile([64, 3, 512], BF16, name="rf_u3")
        rlast3 = nrm.tile([64, 3, 512], BF16, name="rl_u3")
        wt3 = wsm.tile([P, 1, 1, 3, 3, 64], BF16, name="wtu3", tag="wshared",
                       bufs=1)
        nc.sync.dma_start(out=wt3, in_=wu[3][:, :, :, :, :, :])
        for s_i in range(nstrip3):
            y0 = s_i * nr3
            i_lo = y0 // 2
            rows_in = nr3 // 2 + 1
            slab = sb.tile([P, rows_in, Wp3], BF16, name="slu3", tag="inslab")
            nc.sync.dma_start(
                out=slab.rearrange("k r w -> k (r w)"),
                in_=_dap(g2, i_lo * Wp3, [[129 * 257, P], [1, rows_in * Wp3]]))
            rha = 1 if s_i == nstrip3 - 1 else 0
            nc.scalar.activation(out=slab[:, :rows_in - rha, 0:Wi3],
                                 in_=slab[:, :rows_in - rha, 0:Wi3],
                                 func=AF.Relu,
                                 bias=st_g2[0][1], scale=st_g2[0][0])
            osE = sb.tile([64, 2, 512], BF16, name="osE", tag="outslab")
            osO = sb.tile([64, 2, 512], BF16, name="osO", tag="outslab2")
            for a in range(2):
                kys = [1] if a == 0 else [0, 2]
                dst_t = osO if a == 0 else osE  # padded parity = (a+1)%2
                for b_ in range(2):
                    kxs = [1] if b_ == 0 else [0, 2]
                    pt = ps.tile([64, rsub3, Wi3], F32, name="ptu3", tag="mm")
                    first = True
                    for ky in kys:
                        i_first = (y0 + a + 1 - ky) // 2
                        for kx in kxs:
                            j0 = (b_ + 1 - kx) // 2
                            rhs = _ap(slab[:, 0, 0],
                                      (i_first - i_lo) * Wp3 + j0,
                                      [[Wp3, rsub3], [1, Wi3]])
                            last = (ky == kys[-1] and kx == kxs[-1])
                            nc.tensor.matmul(pt, wt3[:, 0, 0, ky, kx, :], rhs,
                                             start=first, stop=last)
                            first = False
                    nc.vector.bn_stats(out=stt3[:, s_i * 4 + a * 2 + b_, :],
                                       in_=pt.rearrange("p a b -> p (a b)"))
                    oap = _ap(dst_t[:, 0, 0], b_, [[512, rsub3], [2, Wi3]])
                    nc.scalar.activation(out=oap, in_=pt, func=AF.Copy)
            if s_i == 0:
                # first3 = padded rows 3,4,5 = osO[0], osE[0], osO[1]
                nc.vector.tensor_copy(out=rfirst3[:, 0, :], in_=osO[:, 0, :])
                nc.vector.tensor_copy(out=rfirst3[:, 1, :], in_=osE[:, 0, :])
                nc.vector.tensor_copy(out=rfirst3[:, 2, :], in_=osO[:, 1, :])
            if s_i == nstrip3 - 1:
                # last3 = padded rows 256,257,258 = osE[0], osO[1], osE[1]
                nc.vector.tensor_copy(out=rlast3[:, 0, :], in_=osE[:, 0, :])
                nc.vector.tensor_copy(out=rlast3[:, 1, :], in_=osO[:, 1, :])
                nc.vector.tensor_copy(out=rlast3[:, 2, :], in_=osE[:, 1, :])
            # store: osE rows k -> g3e idx y0/2+2+k; osO rows k -> g3o y0/2+1+k
            nc.sync.dma_start(
                out=_dap(g3e, (y0 // 2 + 2) * 512, [[131 * 512, 64], [1, 1024]]),
                in_=osE.rearrange("p r w -> p (r w)"))
            nc.sync.dma_start(
                out=_dap(g3o, (y0 // 2 + 1) * 512, [[131 * 512, 64], [1, 1024]]),
                in_=osO.rearrange("p r w -> p (r w)"))
        mv3 = stp.tile([64, 2], F32, name="mvu3", tag="mv")
        nc.vector.bn_aggr(out=mv3, in_=stt3)
        me3 = stp.tile([64, 1, 2], F32, name="meu3", tag="meu3")
        mv_to_me(mv3, me3, 0)
        cc_exchange("b8", me3,
                    [(rfirst3.rearrange("p a w -> p (a w)"), 4),
                     (rlast3.rearrange("p a w -> p (a w)"), 4 + 3 * 512)], 64)
        stg3_t, st_g3p = finalize_stats_batch(
            cc_stats("b8", 1, 64, "g3"), 64, 1, "g3")
        # g3 padded boundary rows:
        #  above (padded 0,1,2): top = reflect own (6,5,4); bottom = peer last3
        #  below (padded 259,260,261): top = peer first3; bottom = reflect own
        #  (257,256,255)
        refl_src = [(g3e, 3), (g3o, 2), (g3e, 2)]        # for above
        refl_dst = [(g3e, 0), (g3o, 0), (g3e, 1)]
        refl_src_b = [(g3o, 128), (g3e, 128), (g3o, 127)]  # for below
        refl_dst_b = [(g3o, 129), (g3e, 130), (g3o, 130)]
        peer_ab = cc_halo("b8", 0, 64, 3 * 512, 4 + 3 * 512, False, maskb)
        peer_bl = cc_halo("b8", 0, 64, 3 * 512, 4, True, maskt)
        for k in range(3):
            for (srcs, dsts, peer_t, mask_own) in (
                    (refl_src, refl_dst, peer_ab, maskt),
                    (refl_src_b, refl_dst_b, peer_bl, maskb)):
                st_, si_ = srcs[k]
                dt_, di_ = dsts[k]
                own = sb.tile([64, 512], BF16, name="g3own", tag="halo2")
                nc.gpsimd.dma_start(
                    out=own, in_=_dap(st_, si_ * 512, [[131 * 512, 64], [1, 512]]))
                nc.vector.tensor_scalar(out=own, in0=own, scalar1=mask_own[:64],
                                        scalar2=None, op0=ALU.mult)
                nc.vector.tensor_add(out=own, in0=own,
                                     in1=peer_t[:, 0, k * 512:(k + 1) * 512])
                nc.gpsimd.dma_start(
                    out=_dap(dt_, di_ * 512, [[131 * 512, 64], [1, 512]]),
                    in_=own)

        nc.leave_named_scope("up", _sc, False)
        _sc = nc.enter_named_scope("final", False)[0]

        # ================= final conv 7x7, 64 -> 3, tanh ==================
        sF128 = nrm.tile([P, 2], F32, name="sF128")
        nc.sync.dma_start(out=sF128[0:64, :], in_=stg3_t[:, 0, :])
        nc.sync.dma_start(out=sF128[64:128, :], in_=stg3_t[:, 0, :])
        wfAt = wsm.tile([P, 6, 126], BF16, name="wfAt", tag="wfA", bufs=1)
        nc.sync.dma_start(out=wfAt, in_=wfA[:, :, :])
        wfSt = wsm.tile([126, 7, 18], BF16, name="wfSt", tag="wfS", bufs=1)
        nc.sync.dma_start(out=wfSt, in_=wfS[:, :, :])
        bft = wsm.tile([18, 1], F32, name="bft", tag="bft", bufs=1)
        nc.sync.dma_start(out=bft, in_=bfv[:, :])
        edgebuf = stp.tile([126, 43, 14], BF16, name="edgebuf", tag="edge")

        for si, y0 in enumerate(Y0LIST):
            slab = sb.tile([P, 6, 512], BF16, name="slF", tag="inslab")
            nc.sync.dma_start(
                out=slab[0:64, :, :].rearrange("p r w -> p (r w)"),
                in_=_dap(g3e, (y0 // 2) * 512, [[131 * 512, 64], [1, 6 * 512]]))
            nc.sync.dma_start(
                out=slab[64:128, :, :].rearrange("p r w -> p (r w)"),
                in_=_dap(g3o, (y0 // 2) * 512, [[131 * 512, 64], [1, 6 * 512]]))
            nc.scalar.activation(out=slab, in_=slab, func=AF.Relu,
                                 bias=sF128[:, 1:2], scale=sF128[:, 0:1])
            pA = psf.tile([126, 512], F32, name="pA", tag="fa")
            for t in range(6):
                rhs = _ap(slab[:, 0, 0], t * 512, [[1, 512]])
                nc.tensor.matmul(pA, wfAt[:, t, :], rhs,
                                 start=(t == 0), stop=(t == 5))
            stg = sb.tile([126, 512], BF16, name="stg", tag="outslab")
            nc.scalar.activation(out=stg, in_=pA, func=AF.Copy)
            nc.vector.tensor_copy(out=edgebuf[:, si, 0:7], in_=stg[:, 0:7])
            nc.vector.tensor_copy(out=edgebuf[:, si, 7:14], in_=stg[:, 505:512])
            pB = psf.tile([18, 506], F32, name="pB", tag="fb", bufs=1)
            for dx in range(7):
                nc.tensor.matmul(pB, wfSt[:, dx, :], stg[:, dx:dx + 506],
                                 start=(dx == 0), stop=(dx == 6))
            ftile = sb.tile([18, 506], F32, name="ftile", tag="ftile")
            nc.scalar.activation(out=ftile, in_=pB, func=AF.Tanh,
                                 bias=bft, scale=1.0)
            nc.sync.dma_start(
                out=_dap(hfout, si * 506, [[43 * 506, 18], [1, 506]]),
                in_=ftile)

        # edge columns: out col x in {0,1,2, 509,510,511}
        pe_ = psf.tile([18, 6, 43], F32, name="pe", tag="fe", bufs=1)
        for xi, x in enumerate([0, 1, 2, 509, 510, 511]):
            for dx in range(7):
                n = x + dx - 3
                if n < 0:
                    n = -n
                elif n > 511:
                    n = 1022 - n
                col = n if n <= 6 else n - 498
                rhs = _ap(edgebuf[:, 0, 0], col, [[14, 43]])
                nc.tensor.matmul(pe_[:, xi, :], wfSt[:, dx, :], rhs,
                                 start=(dx == 0), stop=(dx == 6))
        eft = sb.tile([18, 6, 43], F32, name="eft", tag="ftile")
        nc.scalar.activation(out=eft, in_=pe_, func=AF.Tanh, bias=bft,
                             scale=1.0)
        nc.sync.dma_start(
            out=_dap(efout, 0, [[6 * 43, 18], [1, 6 * 43]]),
            in_=eft.rearrange("p a b -> p (a b)"))
        nc.leave_named_scope("final", _sc, False)

        if debug:
            for nm, tens in [("h0", h0), ("h1", h1), ("h2", h2), ("h3", h3),
                             ("h4", h4), ("g0", g0), ("g1", g1), ("g2", g2),
                             ("g3e", g3e), ("g3o", g3o)]:
                sh = tens.shape
                if len(sh) == 4:
                    nblk, npart, nfree = sh[0], sh[1], sh[2] * sh[3]
                else:
                    nblk, npart, nfree = 1, sh[0], sh[1] * sh[2]
                dbg = nc.dram_tensor("dbg_" + nm, [nblk, npart, nfree], BF16,
                                     kind="ExternalOutput")
                for blk in range(nblk):
                    nc.sync.dma_start(
                        out=_dap(dbg, blk * npart * nfree,
                                 [[nfree, npart], [1, nfree]]),
                        in_=_dap(tens, blk * npart * nfree,
                                 [[nfree, npart], [1, nfree]]))

    nc.finalize()
    return nc


# ----------------------------------------------------------------------------
# Host driver
# ----------------------------------------------------------------------------

def assemble_output(results, inst):
    """Build the full [B,3,H,W] output from per-core hfout/efout."""
    out = np.zeros((B, 3, H, W), np.float32)
    edge_cols = [0, 1, 2, 509, 510, 511]
    for b in range(B):
        hf = np.zeros((3, H, W), np.float32)
        for core, rowbase in ((b, 0), (b + 4, 256)):
            ft = np.asarray(results[core]["hfout"], np.float32)  # [18,43,506]
            ef = np.asarray(results[core]["efout"], np.float32)  # [18,6,43]
            for si, y0 in enumerate(Y0LIST):
                for r in range(6):
                    y = rowbase + y0 + r
                    hf[:, y, 3:509] = ft[r * 3:(r + 1) * 3, si, :]
                    for xi, x in enumerate(edge_cols):
                        hf[:, y, x] = ef[r * 3:(r + 1) * 3, xi, si]
        mask = (np.asarray(inst[b, 0]) == 1)
        cnt = float(mask.sum())
        mean = (hf * mask[None]).sum((1, 2)) / cnt
        out[b] = mean[:, None, None] * mask[None].astype(np.float32)
    return out


_CACHE = {}


def run(inputs, trace=False):
    if "nc" not in _CACHE:
        _CACHE["nc"] = build_kernel()
    nc = _CACHE["nc"]
    wblobs = prep_weights(inputs)
    x = np.asarray(inputs["x"], np.float32)
    in_maps = [prep_core_inputs(x[c % B], c >= B, wblobs) for c in range(8)]
    res = run_bass_kernel_spmd(nc, in_maps, core_ids=list(range(8)),
                               trace=trace)
    return res


def kernel(**inputs):
    res = run(inputs)
    return assemble_output(res.results, np.asarray(inputs["inst"]))
